# revision 24
# baseline (speedup 1.0000x reference)
"""Trainium2 Bass kernel for nn_Attention_23218593202595.

reference:
    hidden = concat([static, dynamic, broadcast(decoder)], axis=1)   # [B, 3H, S]
    u      = tanh(einsum('hk,bks->bhs', W[0], hidden))               # [B, H, S]
    scores = einsum('h,bhs->bs', v[0,0], u)[:, None, :]              # [B, 1, S]
    out    = softmax(scores, axis=2)

B=256, H=256, S=512.  Pure data parallel over 8 NeuronCores: core i owns
batches [32i, 32i+32).  W/v/decoder-projection are tiny and replicated.

Per core, per batch b (q = b%4, one PSUM score bank per 4-batch group):
    psum_u[mc]          = sum_kc Wt[kc, mc]^T @ x[kc]   (x = [static;dynamic], bf16)
    u[mc]               = tanh(psum_u[mc] + c[:, b])    (ScalarE, c = W_dec @ dec)
    sbank[32q:32q+32]  += (v ox 1_32)[mc]^T @ u[mc]     (M=32 matmul -> 32 equal rows,
                                                         pipelined 1 batch behind)
After each 4-batch group the softmax runs directly on the psum bank
(reduce-max -> exp w/ fused sum -> reciprocal -> scale) and rows
{0,32,64,96} are DMAed to the output.

All inputs are converted to bf16 and pre-swizzled on the host into
partition-major layouts so every DMA lands as 128 contiguous runs.
"""
import sys

if "/opt/trn_rl_repo" not in sys.path:
    sys.path.insert(0, "/opt/trn_rl_repo")

import numpy as np
import ml_dtypes

B, H, S = 256, 256, 512
N_CORES = 8
NB = B // N_CORES                    # batches per core
GROUPS = [1, 1, 2] + [4] * 7         # batches per dma group (sum = NB)

_cache = {}


def _build():
    import concourse.bass as bass
    import concourse.mybir as mybir
    import concourse.tile as tile
    from concourse import bacc

    f32 = mybir.dt.float32
    bf16 = mybir.dt.bfloat16
    AF = mybir.ActivationFunctionType

    nc = bacc.Bacc("TRN2", target_bir_lowering=False, debug=False,
                   num_devices=N_CORES)

    # all host-preswizzled, partition-major; big tensors laid out per
    # partition as [b, c, s] so any batch range is one contiguous run
    st_ext = nc.declare_dram_parameter("static", [128, NB * 2 * S], bf16, isOutput=False)
    dy_ext = nc.declare_dram_parameter("dynamic", [128, NB * 2 * S], bf16, isOutput=False)
    de_ext = nc.declare_dram_parameter("dect", [128, 2 * NB], bf16, isOutput=False)
    wt_ext = nc.declare_dram_parameter("wt", [128, 6 * H], bf16, isOutput=False)
    vv_ext = nc.declare_dram_parameter("vv", [128, 2 * 32], bf16, isOutput=False)
    out_ext = nc.declare_dram_parameter("out", [NB, S], f32, isOutput=True)

    with tile.TileContext(nc) as tc:
        with (
            tc.tile_pool(name="const", bufs=1) as cpool,
            tc.tile_pool(name="inp", bufs=4) as ipool,
            tc.tile_pool(name="upool", bufs=14) as upool,
            tc.tile_pool(name="smpool", bufs=2) as smpool,
            tc.tile_pool(name="ps_u", bufs=6, space=bass.MemorySpace.PSUM) as ps_u,
            tc.tile_pool(name="ps_s", bufs=2, space=bass.MemorySpace.PSUM) as ps_s,
        ):
            # ---- constants ----
            # wt first on the sync queue: everything on the PE waits for it
            wt_sb = cpool.tile([128, 6, H], bf16)      # [p, kc, m]
            nc.sync.dma_start(out=wt_sb[:], in_=wt_ext.ap())
            de_sb = cpool.tile([128, 2, NB], bf16)     # [p, c, b]
            vv_sb = cpool.tile([128, 2, 32], bf16)     # [p, c, m] v replicated x32

            # PE warmup while DMAs land: ~3us of tiny matmuls on a zeroed
            # tile so the HAM clock gate opens before the real stream starts
            warm = cpool.tile([128, 64], bf16)
            nc.gpsimd.memset(warm[:], 0.0)
            wpsum = ps_s.tile([64, 64], f32, tag="sps")
            for _ in range(80):
                nc.tensor.matmul(wpsum[:], warm[:, :64], warm[:])

            cbias = cpool.tile([128, 2, NB], f32)

            def emit_cmms():
                # c = W_dec @ dec^T  -> [H, NB] f32, kept as tanh bias
                for mc in range(2):
                    pc = ps_s.tile([128, NB], f32, tag="sps")
                    for kc in range(2):
                        nc.tensor.matmul(
                            pc[:],
                            wt_sb[:, 4 + kc, mc * 128:(mc + 1) * 128],
                            de_sb[:, kc, :],
                            start=(kc == 0), stop=(kc == 1),
                        )
                    nc.vector.tensor_copy(cbias[:, mc, :], pc[:])

            def emit_group_scores(g, items):
                # items: [(b, [u_mc0, u_mc1]) x4] for batches 4g..4g+3.
                # 8 back-to-back M=32 matmuls fill one psum bank with the
                # 4 score rows replicated across each 32-partition slice.
                sbank = ps_s.tile([128, S], f32, tag="sps")
                for q, (b, u_list) in enumerate(items):
                    for mc in range(2):
                        nc.tensor.matmul(sbank[32 * q:32 * (q + 1), :],
                                         vv_sb[:, mc, :], u_list[mc][:],
                                         start=(mc == 0), stop=(mc == 1),
                                         tile_position=(0, 32 * q))
                # softmax for batches 4g..4g+3 straight off the psum bank
                negmax = smpool.tile([128, 1], f32, tag="nm")
                nc.vector.tensor_reduce(negmax[:], sbank[:],
                                        axis=bass.mybir.AxisListType.X,
                                        op=bass.mybir.AluOpType.max,
                                        negate=True)
                prob = smpool.tile([128, S], f32, tag="pr")
                sums = smpool.tile([128, 1], f32, tag="sm")
                nc.scalar.activation(prob[:], sbank[:], AF.Exp,
                                     bias=negmax[:], accum_out=sums[:])
                recip = smpool.tile([128, 1], f32, tag="rc")
                nc.vector.reciprocal(recip[:], sums[:])
                outp = smpool.tile([128, S], f32, tag="op")
                nc.vector.tensor_scalar_mul(outp[:], prob[:], recip[:])
                nc.sync.dma_start(
                    out=out_ext.ap()[4 * g:4 * (g + 1), :],
                    in_=outp[:].rearrange("(q r) s -> q r s", r=32)[:, 0, :])

            # ---- main loop over batches ----
            pending = []
            b0 = 0
            for gi, gsz in enumerate(GROUPS):
                st_t = ipool.tile([128, gsz, 2, S], bf16, tag="st")
                nc.sync.dma_start(
                    out=st_t[:],
                    in_=st_ext.ap()[:, b0 * 2 * S:(b0 + gsz) * 2 * S])
                dy_t = ipool.tile([128, gsz, 2, S], bf16, tag="dy")
                nc.sync.dma_start(
                    out=dy_t[:],
                    in_=dy_ext.ap()[:, b0 * 2 * S:(b0 + gsz) * 2 * S])
                if gi == 0:
                    # small consts after the first input group; needed by the
                    # c-matmuls (emitted after batch 0's mains) and first tanh
                    nc.sync.dma_start(out=de_sb[:], in_=de_ext.ap())
                    nc.sync.dma_start(out=vv_sb[:], in_=vv_ext.ap())

                for j in range(gsz):
                    b = b0 + j
                    psu_list = []
                    for mc in range(2):
                        psu = ps_u.tile([128, S], f32, tag="ups")
                        for kc in range(2):
                            nc.tensor.matmul(
                                psu[:],
                                wt_sb[:, kc, mc * 128:(mc + 1) * 128],
                                st_t[:, j, kc, :],
                                start=(kc == 0), stop=False,
                            )
                        for kc in range(2):
                            nc.tensor.matmul(
                                psu[:],
                                wt_sb[:, 2 + kc, mc * 128:(mc + 1) * 128],
                                dy_t[:, j, kc, :],
                                start=False, stop=(kc == 1),
                            )
                        psu_list.append(psu)
                    if b == 0:
                        emit_cmms()   # cbias must be written before first tanh
                    u_list = []
                    for mc in range(2):
                        u_bf = upool.tile([128, S], bf16, tag="u")
                        nc.scalar.activation(u_bf[:], psu_list[mc][:], AF.Tanh,
                                             bias=cbias[:, mc, b:b + 1])
                        u_list.append(u_bf)
                    # group score matmuls run 2+ batches behind the mains
                    pending.append((b, u_list))
                    if len(pending) >= 6:
                        g = pending[0][0] // 4
                        emit_group_scores(g, pending[:4])
                        pending = pending[4:]
                b0 += gsz
            while pending:
                g = pending[0][0] // 4
                emit_group_scores(g, pending[:4])
                pending = pending[4:]

    nc.compile()
    return nc


def _get_nc():
    if "nc" not in _cache:
        _cache["nc"] = _build()
    return _cache["nc"]


def make_in_maps(static_hidden, dynamic_hidden, decoder_hidden, W, v):
    bf = ml_dtypes.bfloat16
    # W[0] is [H, 3H]; wt[p, kc*H + m] = W[0][m, kc*128 + p]
    wt = np.ascontiguousarray(
        W[0].T.astype(bf).reshape(6, 128, H).transpose(1, 0, 2).reshape(128, 6 * H))
    # v replicated 32x: vv[p, c*32 + m] = v[c*128 + p]
    vv = np.ascontiguousarray(
        np.repeat(v[0, 0].astype(bf).reshape(2, 128).T[:, :, None], 32, axis=2)
        .reshape(128, 64))

    def swizzle_big(x):
        # [NB, H, S] -> [128, NB*2*S]; h = c*128 + p; free layout [b, c, s]
        return np.ascontiguousarray(
            x.astype(bf).reshape(NB, 2, 128, S)
            .transpose(2, 0, 1, 3).reshape(128, NB * 2 * S))

    in_maps = []
    for i in range(N_CORES):
        sl = slice(i * NB, (i + 1) * NB)
        dect = np.ascontiguousarray(
            decoder_hidden[sl].T.astype(bf).reshape(2, 128, NB)
            .transpose(1, 0, 2).reshape(128, 2 * NB))
        in_maps.append({
            "static": swizzle_big(static_hidden[sl]),
            "dynamic": swizzle_big(dynamic_hidden[sl]),
            "dect": dect,
            "wt": wt,
            "vv": vv,
        })
    return in_maps


def kernel(static_hidden, dynamic_hidden, decoder_hidden, W, v):
    from concourse.bass_utils import run_bass_kernel_spmd

    nc = _get_nc()
    in_maps = make_in_maps(static_hidden, dynamic_hidden, decoder_hidden, W, v)
    res = run_bass_kernel_spmd(nc, in_maps, list(range(N_CORES)))
    out = np.concatenate([res.results[i]["out"] for i in range(N_CORES)], axis=0)
    return out[:, None, :].astype(np.float32)


# revision 26
# speedup vs baseline: 1.0414x; 1.0414x over previous
"""Trainium2 Bass kernel for nn_Attention_23218593202595.

reference:
    hidden = concat([static, dynamic, broadcast(decoder)], axis=1)   # [B, 3H, S]
    u      = tanh(einsum('hk,bks->bhs', W[0], hidden))               # [B, H, S]
    scores = einsum('h,bhs->bs', v[0,0], u)[:, None, :]              # [B, 1, S]
    out    = softmax(scores, axis=2)

B=256, H=256, S=512.  Pure data parallel over 8 NeuronCores: core i owns
batches [32i, 32i+32).  W/v/decoder-projection are tiny and replicated.

Per core, per batch b (q = b%4, one PSUM score bank per 4-batch group):
    psum_u[mc]          = sum_kc Wt[kc, mc]^T @ x[kc]   (x = [static;dynamic], bf16)
    u[mc]               = tanh(psum_u[mc] + c[:, b])    (ScalarE, c = W_dec @ dec)
    sbank[32q:32q+32]  += (v ox 1_32)[mc]^T @ u[mc]     (M=32 matmul -> 32 equal rows,
                                                         pipelined 1 batch behind)
After each 4-batch group the softmax runs directly on the psum bank
(reduce-max -> exp w/ fused sum -> reciprocal -> scale) and rows
{0,32,64,96} are DMAed to the output.

All inputs are converted to bf16 and pre-swizzled on the host into
partition-major layouts so every DMA lands as 128 contiguous runs.
"""
import sys

if "/opt/trn_rl_repo" not in sys.path:
    sys.path.insert(0, "/opt/trn_rl_repo")

import numpy as np
import ml_dtypes

B, H, S = 256, 256, 512
N_CORES = 8
NB = B // N_CORES                    # batches per core
GROUPS = [1, 1, 2] + [4] * 7         # batches per dma group (sum = NB)

_cache = {}


def _build():
    import concourse.bass as bass
    import concourse.mybir as mybir
    import concourse.tile as tile
    from concourse import bacc

    f32 = mybir.dt.float32
    bf16 = mybir.dt.bfloat16
    AF = mybir.ActivationFunctionType

    nc = bacc.Bacc("TRN2", target_bir_lowering=False, debug=False,
                   num_devices=N_CORES)

    # all host-preswizzled, partition-major; big tensors laid out per
    # partition as [b, c, s] so any batch range is one contiguous run
    st_ext = nc.declare_dram_parameter("static", [128, NB * 2 * S], bf16, isOutput=False)
    dy_ext = nc.declare_dram_parameter("dynamic", [128, NB * 2 * S], bf16, isOutput=False)
    de_ext = nc.declare_dram_parameter("dect", [128, 2 * NB], bf16, isOutput=False)
    wt_ext = nc.declare_dram_parameter("wt", [128, 6 * H], bf16, isOutput=False)
    vv_ext = nc.declare_dram_parameter("vv", [128, 2 * 32], bf16, isOutput=False)
    out_ext = nc.declare_dram_parameter("out", [NB, S], f32, isOutput=True)

    with tile.TileContext(nc) as tc:
        with (
            tc.tile_pool(name="const", bufs=1) as cpool,
            tc.tile_pool(name="inp", bufs=4) as ipool,
            tc.tile_pool(name="upool", bufs=17) as upool,
            tc.tile_pool(name="smpool", bufs=2) as smpool,
            tc.tile_pool(name="ps_u", bufs=6, space=bass.MemorySpace.PSUM) as ps_u,
            tc.tile_pool(name="ps_s", bufs=2, space=bass.MemorySpace.PSUM) as ps_s,
        ):
            # ---- constants ----
            # wt first on the sync queue: everything on the PE waits for it
            wt_sb = cpool.tile([128, 6, H], bf16)      # [p, kc, m]
            nc.sync.dma_start(out=wt_sb[:], in_=wt_ext.ap())
            de_sb = cpool.tile([128, 2, NB], bf16)     # [p, c, b]
            vv_sb = cpool.tile([128, 2, 32], bf16)     # [p, c, m] v replicated x32

            # PE warmup while DMAs land: ~3us of tiny matmuls on a zeroed
            # tile so the HAM clock gate opens before the real stream starts
            warm = cpool.tile([128, 64], bf16)
            nc.gpsimd.memset(warm[:], 0.0)
            wpsum = ps_s.tile([64, 64], f32, tag="sps")
            for _ in range(80):
                nc.tensor.matmul(wpsum[:], warm[:, :64], warm[:])

            cbias = cpool.tile([128, 2, NB], f32)

            def emit_cmms():
                # c = W_dec @ dec^T  -> [H, NB] f32, kept as tanh bias
                for mc in range(2):
                    pc = ps_s.tile([128, NB], f32, tag="sps")
                    for kc in range(2):
                        nc.tensor.matmul(
                            pc[:],
                            wt_sb[:, 4 + kc, mc * 128:(mc + 1) * 128],
                            de_sb[:, kc, :],
                            start=(kc == 0), stop=(kc == 1),
                        )
                    nc.vector.tensor_copy(cbias[:, mc, :], pc[:])

            def emit_group_scores(g, items):
                # items: [(b, [u_mc0, u_mc1]) x4] for batches 4g..4g+3.
                # 8 back-to-back M=32 matmuls fill one psum bank with the
                # 4 score rows replicated across each 32-partition slice.
                sbank = ps_s.tile([128, S], f32, tag="sps")
                for q, (b, u_list) in enumerate(items):
                    for mc in range(2):
                        nc.tensor.matmul(sbank[32 * q:32 * (q + 1), :],
                                         vv_sb[:, mc, :], u_list[mc][:],
                                         start=(mc == 0), stop=(mc == 1),
                                         tile_position=(0, 32 * q))
                # softmax for batches 4g..4g+3 straight off the psum bank
                negmax = smpool.tile([128, 1], f32, tag="nm")
                nc.vector.tensor_reduce(negmax[:], sbank[:],
                                        axis=bass.mybir.AxisListType.X,
                                        op=bass.mybir.AluOpType.max,
                                        negate=True)
                prob = smpool.tile([128, S], f32, tag="pr")
                sums = smpool.tile([128, 1], f32, tag="sm")
                nc.scalar.activation(prob[:], sbank[:], AF.Exp,
                                     bias=negmax[:], accum_out=sums[:])
                recip = smpool.tile([128, 1], f32, tag="rc")
                nc.vector.reciprocal(recip[:], sums[:])
                outp = smpool.tile([128, S], f32, tag="op")
                nc.vector.tensor_scalar_mul(outp[:], prob[:], recip[:])
                nc.sync.dma_start(
                    out=out_ext.ap()[4 * g:4 * (g + 1), :],
                    in_=outp[:].rearrange("(q r) s -> q r s", r=32)[:, 0, :])

            # ---- main loop over batches ----
            pending = []
            b0 = 0
            for gi, gsz in enumerate(GROUPS):
                st_t = ipool.tile([128, gsz, 2, S], bf16, tag="st")
                nc.sync.dma_start(
                    out=st_t[:],
                    in_=st_ext.ap()[:, b0 * 2 * S:(b0 + gsz) * 2 * S])
                dy_t = ipool.tile([128, gsz, 2, S], bf16, tag="dy")
                nc.sync.dma_start(
                    out=dy_t[:],
                    in_=dy_ext.ap()[:, b0 * 2 * S:(b0 + gsz) * 2 * S])
                if gi == 0:
                    # small consts after the first input group; needed by the
                    # c-matmuls (emitted after batch 0's mains) and first tanh
                    nc.sync.dma_start(out=de_sb[:], in_=de_ext.ap())
                    nc.sync.dma_start(out=vv_sb[:], in_=vv_ext.ap())

                for j in range(gsz):
                    b = b0 + j
                    psu_list = []
                    for mc in range(2):
                        psu = ps_u.tile([128, S], f32, tag="ups")
                        for kc in range(2):
                            nc.tensor.matmul(
                                psu[:],
                                wt_sb[:, kc, mc * 128:(mc + 1) * 128],
                                st_t[:, j, kc, :],
                                start=(kc == 0), stop=False,
                            )
                        for kc in range(2):
                            nc.tensor.matmul(
                                psu[:],
                                wt_sb[:, 2 + kc, mc * 128:(mc + 1) * 128],
                                dy_t[:, j, kc, :],
                                start=False, stop=(kc == 1),
                            )
                        psu_list.append(psu)
                    if b == 0:
                        emit_cmms()   # cbias must be written before first tanh
                    u_list = []
                    for mc in range(2):
                        u_bf = upool.tile([128, S], bf16, tag="u")
                        nc.scalar.activation(u_bf[:], psu_list[mc][:], AF.Tanh,
                                             bias=cbias[:, mc, b:b + 1])
                        u_list.append(u_bf)
                    # group score matmuls run 2+ batches behind the mains
                    pending.append((b, u_list))
                    if len(pending) >= 5:
                        g = pending[0][0] // 4
                        emit_group_scores(g, pending[:4])
                        pending = pending[4:]
                b0 += gsz
            while pending:
                g = pending[0][0] // 4
                emit_group_scores(g, pending[:4])
                pending = pending[4:]

    nc.compile()
    return nc


def _get_nc():
    if "nc" not in _cache:
        _cache["nc"] = _build()
    return _cache["nc"]


def make_in_maps(static_hidden, dynamic_hidden, decoder_hidden, W, v):
    bf = ml_dtypes.bfloat16
    # W[0] is [H, 3H]; wt[p, kc*H + m] = W[0][m, kc*128 + p]
    wt = np.ascontiguousarray(
        W[0].T.astype(bf).reshape(6, 128, H).transpose(1, 0, 2).reshape(128, 6 * H))
    # v replicated 32x: vv[p, c*32 + m] = v[c*128 + p]
    vv = np.ascontiguousarray(
        np.repeat(v[0, 0].astype(bf).reshape(2, 128).T[:, :, None], 32, axis=2)
        .reshape(128, 64))

    def swizzle_big(x):
        # [NB, H, S] -> [128, NB*2*S]; h = c*128 + p; free layout [b, c, s]
        return np.ascontiguousarray(
            x.astype(bf).reshape(NB, 2, 128, S)
            .transpose(2, 0, 1, 3).reshape(128, NB * 2 * S))

    in_maps = []
    for i in range(N_CORES):
        sl = slice(i * NB, (i + 1) * NB)
        dect = np.ascontiguousarray(
            decoder_hidden[sl].T.astype(bf).reshape(2, 128, NB)
            .transpose(1, 0, 2).reshape(128, 2 * NB))
        in_maps.append({
            "static": swizzle_big(static_hidden[sl]),
            "dynamic": swizzle_big(dynamic_hidden[sl]),
            "dect": dect,
            "wt": wt,
            "vv": vv,
        })
    return in_maps


def kernel(static_hidden, dynamic_hidden, decoder_hidden, W, v):
    from concourse.bass_utils import run_bass_kernel_spmd

    nc = _get_nc()
    in_maps = make_in_maps(static_hidden, dynamic_hidden, decoder_hidden, W, v)
    res = run_bass_kernel_spmd(nc, in_maps, list(range(N_CORES)))
    out = np.concatenate([res.results[i]["out"] for i in range(N_CORES)], axis=0)
    return out[:, None, :].astype(np.float32)
